# revision 2
# baseline (speedup 1.0000x reference)
"""Multi-head attention (B=4, S=2048, DM=1024, H=16, DH=64) on 8 TRN2 cores.

Sharding: 8 cores = 4 batches x 2 head-halves. Core c handles batch c//2 and
heads [ (c%2)*8, (c%2)*8+8 ).  Each core projects Q/K/V for its 8 heads,
runs causal softmax attention, applies its slice of w_o, and writes a
partial [S, DM] output.  The host sums the two partials per batch.

Attention uses a transposed-PV formulation: exp(logits) is kept in
[kv, q] layout and used as the *stationary* operand of the PV matmuls,
producing head outputs in [q, (head, d)] layout with a fused ones-column
denominator.  After a per-partition reciprocal multiply the [q, d] tiles
are transposed back to [d, q] on the PE (identity-matmul transpose) to
feed w_o.  All matmuls are bf16 with fp32 PSUM accumulation.
"""

import math

import ml_dtypes
import numpy as np

B, S, DM, H, DH = 4, 2048, 1024, 16, 64
NCORES = 8
HPC = H // 2        # heads per core
PAIRS = HPC // 2    # head pairs per core (packed 2-per-128-partitions)
F = 512             # query block (free dim of QK matmuls)
CH = 128            # kv chunk (partition dim of transposed logits)
NQB = S // F        # query blocks
NT = S // CH        # kv chunks
VE = DH + 1         # V extended with a ones column (fused denominator)
KT = DM // 128      # contraction k-tiles for projections
KO = HPC * DH // 128  # contraction k-tiles for w_o
SCALE = 1.0 / math.sqrt(DH)

_CACHE = {}


def _split_excess_waits(nc):
    """This environment's walrus rejects instructions carrying more than one
    sync wait ("Too many sync wait commands").  Hoist excess waits onto
    single-wait NoOps inserted right before the offending instruction."""
    import concourse.mybir as mybir

    n = 0
    for f in nc.m.functions:
        for blk in f.blocks:
            newlist = []
            for ins in blk.instructions:
                si = ins.sync_info
                if si is not None and len(si.on_wait) > 1:
                    for w in si.on_wait[:-1]:
                        n += 1
                        newlist.append(
                            mybir.InstNoOp(
                                name=f"I-waitfix-{n}",
                                opcode="NoOp",
                                engine=ins.engine,
                                sync_info=mybir.SyncInfo(on_wait=[w], on_update=[]),
                            )
                        )
                    si.on_wait = si.on_wait[-1:]
                newlist.append(ins)
            blk.instructions = newlist
    return n


def _build(causal):
    import concourse.bass as bass
    import concourse.mybir as mybir
    import concourse.tile as tile

    bf16 = mybir.dt.bfloat16
    f32 = mybir.dt.float32
    Exp = mybir.ActivationFunctionType.Exp

    nc = bass.Bass()
    et = nc.dram_tensor("et", [DM, S], bf16, kind="ExternalInput")
    wq = nc.dram_tensor("wq", [DM, HPC * DH], bf16, kind="ExternalInput")
    wk = nc.dram_tensor("wk", [DM, HPC * DH], bf16, kind="ExternalInput")
    wv = nc.dram_tensor("wv", [DM, HPC * DH], bf16, kind="ExternalInput")
    wo = nc.dram_tensor("wo", [HPC * DH, DM], bf16, kind="ExternalInput")
    band2 = nc.dram_tensor("band2", [CH, 2 * CH], bf16, kind="ExternalInput")
    ident = nc.dram_tensor("ident", [CH, CH], bf16, kind="ExternalInput")
    out = nc.dram_tensor("out", [S, DM], f32, kind="ExternalOutput")

    with tile.TileContext(nc) as tc:
        with tc.tile_pool(name="const", bufs=1) as cpool, \
             tc.tile_pool(name="qk", bufs=2) as qkpool, \
             tc.tile_pool(name="eexp", bufs=2) as epool, \
             tc.tile_pool(name="heads", bufs=1) as hpool, \
             tc.tile_pool(name="norm", bufs=2) as npool, \
             tc.tile_pool(name="outp", bufs=2) as opool, \
             tc.tile_pool(name="ps", bufs=1, space="PSUM") as ps:

            # ---- input loads.  All DMA transfers serialize on the shared
            # DMA engine pool, so order them so the first V-projection
            # matmul (needs wv kt0 + et chunk-0 kt0) can start ~2us in. ----
            wv_t = cpool.tile([128, KT * HPC * DH], bf16, name="wv_t")
            et_t = cpool.tile([128, KT * S], bf16, name="et_t")
            etr = et.rearrange("(a p) n -> p a n", p=128)
            ett = et_t.rearrange("p (a n) -> p a n", a=KT)
            NQ4 = S // 4
            nc.sync.dma_start(ett[:, :, 0:NQ4], etr[:, :, 0:NQ4])
            for kt in range(KT):
                nc.scalar.dma_start(
                    wv_t[:, kt * HPC * DH : (kt + 1) * HPC * DH],
                    wv[kt * 128 : (kt + 1) * 128, :],
                )
            for cq in range(1, 4):
                nc.sync.dma_start(
                    ett[:, :, cq * NQ4 : (cq + 1) * NQ4],
                    etr[:, :, cq * NQ4 : (cq + 1) * NQ4],
                )
            # weights for the first attention pair early
            w_tiles = {}
            for nm, src in (("wq", wq), ("wk", wk)):
                t = cpool.tile([128, KT * HPC * DH], bf16, name=f"{nm}_t")
                nc.scalar.dma_start(
                    t.rearrange("p (a n) -> p a n", a=KT),
                    src.rearrange("(a p) n -> p a n", p=128),
                )
                w_tiles[nm] = t
            wq_t, wk_t = w_tiles["wq"], w_tiles["wk"]
            wo_t = cpool.tile([128, KO * DM], bf16, name="wo_t")
            nc.sync.dma_start(
                wo_t.rearrange("p (a n) -> p a n", a=KO),
                wo.rearrange("(a p) n -> p a n", p=128),
            )
            band_t = cpool.tile([CH, 2 * CH], bf16, name="band_t")
            nc.sync.dma_start(band_t[:], band2[:])
            id_t = cpool.tile([CH, CH], bf16, name="id_t")
            nc.sync.dma_start(id_t[:], ident[:])

            # ---- V projection: vsb per kv chunk i: [128, 8x(64+ones)] ----
            vsb = cpool.tile([128, NT * HPC * VE], bf16, name="vsb")
            nc.vector.memset(
                vsb.rearrange("p (i e) -> p i e", e=VE)[:, :, DH:VE], 1.0
            )

            def v_chunk(i):
                vps = ps.tile([128, 512], f32, tag="mm512", bufs=2, name="vps")
                for kt in range(KT):
                    nc.tensor.matmul(
                        vps[:],
                        et_t[:, kt * S + i * CH : kt * S + (i + 1) * CH],
                        wv_t[:, kt * HPC * DH : (kt + 1) * HPC * DH],
                        start=(kt == 0),
                        stop=(kt == KT - 1),
                    )
                nc.vector.tensor_copy(
                    vsb[:, i * HPC * VE : (i + 1) * HPC * VE].rearrange(
                        "p (h e) -> p h e", e=VE
                    )[:, :, 0:DH],
                    vps.rearrange("p (h d) -> p h d", d=DH),
                )

            # Q/K projections, one [128, 512] tile at a time (emitted as PE
            # filler work inside earlier attention loops).
            qk_tiles = {}

            def qk_tile(p, wsel, jq):
                if p not in qk_tiles:
                    qt2 = qkpool.tile([128, S], bf16, tag="qt2", name="qt2")
                    kt2 = qkpool.tile([128, S], bf16, tag="kt2", name="kt2")
                    qk_tiles[p] = (qt2, kt2)
                wt = (wq_t, wk_t)[wsel]
                dst = qk_tiles[p][wsel]
                pps = ps.tile([128, 512], f32, tag="mm512", bufs=2, name="pps")
                for kt in range(KT):
                    nc.tensor.matmul(
                        pps[:],
                        wt[:, kt * HPC * DH + p * 128 : kt * HPC * DH + (p + 1) * 128],
                        et_t[:, kt * S + jq * F : kt * S + (jq + 1) * F],
                        start=(kt == 0),
                        stop=(kt == KT - 1),
                    )
                nc.vector.tensor_copy(dst[:, jq * F : (jq + 1) * F], pps[:])

            headsT = [
                hpool.tile([128, S], bf16, name=f"headsT{t}", tag=f"headsT{t}")
                for t in range(PAIRS)
            ]

            # Deferred work (emitted one qb later to hide DVE latency
            # behind the next qb's matmul stream).
            pending = []

            def flush_pending():
                while pending:
                    pending.pop(0)()

            def make_tail(p, qb, normed):
                """Transposes (and for the last pair, w_o) for (p, qb)."""

                def emit():
                    for j in range(NQB):
                        st = qb * NQB + j
                        tps = ps.tile([128, 512], f32, tag="mm512", bufs=2, name="tps")
                        tps_bf = tps.bitcast(bf16)
                        nc.tensor.matmul(
                            tps_bf[:, 0:CH],
                            normed[:, j * CH : (j + 1) * CH],
                            id_t[:],
                            is_transpose=True,
                        )
                        nc.vector.tensor_copy(
                            headsT[p][:, st * CH : (st + 1) * CH], tps_bf[:, 0:CH]
                        )
                        if p == PAIRS - 1:
                            ot = opool.tile([128, DM], f32, tag="ot", name="ot")
                            for nh in range(2):
                                wps = ps.tile([128, 512], f32, tag="mm512", bufs=2, name="wps")
                                for ktt in range(KO):
                                    nc.tensor.matmul(
                                        wps[:],
                                        headsT[ktt][:, st * CH : (st + 1) * CH],
                                        wo_t[:, ktt * DM + nh * 512 : ktt * DM + (nh + 1) * 512],
                                        start=(ktt == 0),
                                        stop=(ktt == KO - 1),
                                    )
                                nc.vector.tensor_copy(ot[:, nh * 512 : (nh + 1) * 512], wps[:])
                                nc.sync.dma_start(
                                    out[st * CH : (st + 1) * CH, nh * 512 : (nh + 1) * 512],
                                    ot[:, nh * 512 : (nh + 1) * 512],
                                )

                return emit

            # PE filler for slot (p, qb): projections needed strictly later.
            def fillers(p, qb):
                if p == 0 and qb < 3:
                    for i in range(4 * qb + 4, 4 * qb + 8):
                        v_chunk(i)
                if qb < 3:
                    qk_tile(p, 0, qb + 1)
                    qk_tile(p, 1, qb + 1)
                elif p + 1 < PAIRS:
                    qk_tile(p + 1, 0, 0)
                    qk_tile(p + 1, 1, 0)

            # lead-in: V chunks + first pair's first q/k tiles
            for i in range(4):
                v_chunk(i)
            qk_tile(0, 0, 0)
            qk_tile(0, 1, 0)

            for p in range(PAIRS):
                qt2, kt2 = qk_tiles[p]
                for qb in range(NQB):
                    nch = 4 * qb + 4 if causal else NT
                    e_grp = epool.tile([128, NT * 2 * F], bf16, tag="e", name="e_grp")

                    # r0: first causally-live query column within this qb
                    # block for chunk c (block-granular band narrowing)
                    def _r0(c):
                        return (c - 4 * qb) * CH if causal and c >= 4 * qb else 0

                    # ---- logits + exp (+ diagonal band mask on Pool) ----
                    for c in range(nch):
                        r0 = _r0(c)
                        stg = ps.tile([128, 2 * F], f32, tag="stg", bufs=2, name="stg")
                        for hh in (0, 1):
                            nc.tensor.matmul(
                                stg[:, hh * F + r0 : (hh + 1) * F],
                                kt2[64 * hh : 64 * hh + 64, c * CH : (c + 1) * CH],
                                qt2[64 * hh : 64 * hh + 64, qb * F + r0 : (qb + 1) * F],
                                start=True,
                                stop=True,
                            )
                        st3 = stg.rearrange("p (h f) -> p h f", h=2)[:, :, r0:F]
                        ex3 = e_grp[:, 2 * c * F : (2 * c + 2) * F].rearrange(
                            "p (h f) -> p h f", h=2
                        )[:, :, r0:F]
                        nc.scalar.activation(ex3, st3, Exp, scale=SCALE)
                        if causal and c >= 4 * qb:
                            # staircase mask on the diagonal 128-col block
                            j = c - 4 * qb
                            sl = e_grp[:, 2 * c * F : (2 * c + 2) * F].rearrange(
                                "p (h f) -> p h f", h=2
                            )[:, :, j * CH : (j + 1) * CH]
                            nc.gpsimd.tensor_mul(
                                sl, sl, band_t.rearrange("p (h f) -> p h f", h=2)
                            )

                    fillers(p, qb)

                    # ---- PV (transposed): out[q, (j2,hh,65)] per j-pair ----
                    # Four accumulation groups (2 qtiles x 2 heads) share each
                    # psum bank.  A matmul with start=True would zero the
                    # whole 2KB region (clobbering sibling groups), so the
                    # bank is zeroed once by an explicit memset and every
                    # matmul accumulates (start=False).
                    pv_t = [
                        ps.tile([128, 512], f32, tag="pv", bufs=2, name="pv")
                        for _ in range(2)
                    ]
                    for pv in pv_t:
                        nc.vector.memset(pv[:, 0 : 4 * VE], 0.0)
                    for jp in range(2):
                        jlo, jhi = 2 * jp, 2 * jp + 1
                        pv = pv_t[jp]
                        clast = (4 * qb + jhi) if causal else NT - 1
                        for c in range(clast + 1):
                            for jloc, j in ((0, jlo), (1, jhi)):
                                cg_last = (4 * qb + j) if causal else NT - 1
                                if c > cg_last:
                                    continue
                                for hh in (0, 1):
                                    nc.tensor.matmul(
                                        pv[:, (jloc * 2 + hh) * VE : (jloc * 2 + hh + 1) * VE],
                                        e_grp[:, (2 * c + hh) * F + j * CH : (2 * c + hh) * F + (j + 1) * CH],
                                        vsb[:, c * HPC * VE + (2 * p + hh) * VE : c * HPC * VE + (2 * p + hh + 1) * VE],
                                        start=False,
                                        stop=(c == cg_last),
                                        skip_group_check=True,
                                    )
                        if jp == 0:
                            normed = npool.tile([128, 512], bf16, tag="normed", name="normed")
                        recip4 = npool.tile([128, 4], f32, tag="recip4", name="recip4")
                        pv4 = pv[:, 0 : 4 * VE].rearrange("p (j h e) -> p j h e", j=2, h=2)
                        with tc.high_priority(offset=400):
                            nc.vector.reciprocal(
                                recip4.rearrange("p (j h o) -> p j h o", j=2, h=2),
                                pv4[:, :, :, DH : DH + 1],
                            )
                            nc.vector.tensor_mul(
                                normed[:, jp * 256 : (jp + 1) * 256].rearrange(
                                    "p (j h e) -> p j h e", j=2, h=2
                                ),
                                pv4[:, :, :, 0:DH],
                                recip4.rearrange("p (j h o) -> p j h o", j=2, h=2)
                                .broadcast_to([128, 2, 2, DH]),
                            )

                    # ---- deferred transposes / w_o from the previous qb ----
                    flush_pending()
                    pending.append(make_tail(p, qb, normed))

            flush_pending()

    _split_excess_waits(nc)
    return nc


# revision 4
# speedup vs baseline: 1.0284x; 1.0284x over previous
"""Multi-head attention (B=4, S=2048, DM=1024, H=16, DH=64) on 8 TRN2 cores.

Sharding: 8 cores = 4 batches x 2 head-halves. Core c handles batch c//2 and
heads [ (c%2)*8, (c%2)*8+8 ).  Each core projects Q/K/V for its 8 heads,
runs causal softmax attention, applies its slice of w_o, and writes a
partial [S, DM] output.  The host sums the two partials per batch.

Attention uses a transposed-PV formulation: exp(logits) is kept in
[kv, q] layout and used as the *stationary* operand of the PV matmuls,
producing head outputs in [q, (head, d)] layout with a fused ones-column
denominator.  After a per-partition reciprocal multiply the [q, d] tiles
are transposed back to [d, q] on the PE (identity-matmul transpose) to
feed w_o.  All matmuls are bf16 with fp32 PSUM accumulation.
"""

import math

import ml_dtypes
import numpy as np

B, S, DM, H, DH = 4, 2048, 1024, 16, 64
NCORES = 8
HPC = H // 2        # heads per core
PAIRS = HPC // 2    # head pairs per core (packed 2-per-128-partitions)
F = 512             # query block (free dim of QK matmuls)
CH = 128            # kv chunk (partition dim of transposed logits)
NQB = S // F        # query blocks
NT = S // CH        # kv chunks
VE = DH + 1         # V extended with a ones column (fused denominator)
KT = DM // 128      # contraction k-tiles for projections
KO = HPC * DH // 128  # contraction k-tiles for w_o
SCALE = 1.0 / math.sqrt(DH)

_CACHE = {}


def _split_excess_waits(nc):
    """This environment's walrus rejects instructions carrying more than one
    sync wait ("Too many sync wait commands").  Hoist excess waits onto
    single-wait NoOps inserted right before the offending instruction."""
    import concourse.mybir as mybir

    n = 0
    for f in nc.m.functions:
        for blk in f.blocks:
            newlist = []
            for ins in blk.instructions:
                si = ins.sync_info
                if si is not None and len(si.on_wait) > 1:
                    for w in si.on_wait[:-1]:
                        n += 1
                        newlist.append(
                            mybir.InstNoOp(
                                name=f"I-waitfix-{n}",
                                opcode="NoOp",
                                engine=ins.engine,
                                sync_info=mybir.SyncInfo(on_wait=[w], on_update=[]),
                            )
                        )
                    si.on_wait = si.on_wait[-1:]
                newlist.append(ins)
            blk.instructions = newlist
    return n


def _build(causal):
    import concourse.bass as bass
    import concourse.mybir as mybir
    import concourse.tile as tile

    bf16 = mybir.dt.bfloat16
    f32 = mybir.dt.float32
    Exp = mybir.ActivationFunctionType.Exp

    nc = bass.Bass()
    et = nc.dram_tensor("et", [DM, S], bf16, kind="ExternalInput")
    wq = nc.dram_tensor("wq", [DM, HPC * DH], bf16, kind="ExternalInput")
    wk = nc.dram_tensor("wk", [DM, HPC * DH], bf16, kind="ExternalInput")
    wv = nc.dram_tensor("wv", [DM, HPC * DH], bf16, kind="ExternalInput")
    wo = nc.dram_tensor("wo", [HPC * DH, DM], bf16, kind="ExternalInput")
    band2 = nc.dram_tensor("band2", [CH, 2 * CH], bf16, kind="ExternalInput")
    ident = nc.dram_tensor("ident", [CH, CH], bf16, kind="ExternalInput")
    out = nc.dram_tensor("out", [S, DM], f32, kind="ExternalOutput")

    with tile.TileContext(nc) as tc:
        with tc.tile_pool(name="const", bufs=1) as cpool, \
             tc.tile_pool(name="qk", bufs=2) as qkpool, \
             tc.tile_pool(name="eexp", bufs=2) as epool, \
             tc.tile_pool(name="heads", bufs=1) as hpool, \
             tc.tile_pool(name="norm", bufs=2) as npool, \
             tc.tile_pool(name="outp", bufs=2) as opool, \
             tc.tile_pool(name="ps", bufs=1, space="PSUM") as ps:

            # ---- input loads.  All DMA transfers serialize on the shared
            # DMA engine pool, so order them so the first V-projection
            # matmul (needs wv kt0 + et chunk-0 kt0) can start ~2us in. ----
            wv_t = cpool.tile([128, KT * HPC * DH], bf16, name="wv_t")
            et_t = cpool.tile([128, KT * S], bf16, name="et_t")
            etr = et.rearrange("(a p) n -> p a n", p=128)
            ett = et_t.rearrange("p (a n) -> p a n", a=KT)
            NQ4 = S // 4
            nc.sync.dma_start(ett[:, :, 0:NQ4], etr[:, :, 0:NQ4])
            for kt in range(KT):
                nc.scalar.dma_start(
                    wv_t[:, kt * HPC * DH : (kt + 1) * HPC * DH],
                    wv[kt * 128 : (kt + 1) * 128, :],
                )
            nc.sync.dma_start(ett[:, :, NQ4 : 2 * NQ4], etr[:, :, NQ4 : 2 * NQ4])
            # weights for the first attention pair; issued on the Act queue
            # AFTER the wv pieces so their DMA-engine service slots land
            # between et chunks (service is FIFO by arrival time).
            w_tiles = {}
            for nm, src in (("wq", wq), ("wk", wk)):
                t = cpool.tile([128, KT * HPC * DH], bf16, name=f"{nm}_t")
                nc.scalar.dma_start(
                    t.rearrange("p (a n) -> p a n", a=KT),
                    src.rearrange("(a p) n -> p a n", p=128),
                )
                w_tiles[nm] = t
            wq_t, wk_t = w_tiles["wq"], w_tiles["wk"]
            for cq in range(2, 4):
                nc.scalar.dma_start(
                    ett[:, :, cq * NQ4 : (cq + 1) * NQ4],
                    etr[:, :, cq * NQ4 : (cq + 1) * NQ4],
                )
            wo_t = cpool.tile([128, KO * DM], bf16, name="wo_t")
            nc.scalar.dma_start(
                wo_t.rearrange("p (a n) -> p a n", a=KO),
                wo.rearrange("(a p) n -> p a n", p=128),
            )
            band_t = cpool.tile([CH, 2 * CH], bf16, name="band_t")
            nc.sync.dma_start(band_t[:], band2[:])
            id_t = cpool.tile([CH, CH], bf16, name="id_t")
            nc.sync.dma_start(id_t[:], ident[:])

            # ---- V projection: vsb per kv chunk i: [128, 8x(64+ones)] ----
            vsb = cpool.tile([128, NT * HPC * VE], bf16, name="vsb")
            nc.vector.memset(
                vsb.rearrange("p (i e) -> p i e", e=VE)[:, :, DH:VE], 1.0
            )

            def v_chunk(i):
                vps = ps.tile([128, 512], f32, tag="mm512", bufs=2, name="vps")
                for kt in range(KT):
                    nc.tensor.matmul(
                        vps[:],
                        et_t[:, kt * S + i * CH : kt * S + (i + 1) * CH],
                        wv_t[:, kt * HPC * DH : (kt + 1) * HPC * DH],
                        start=(kt == 0),
                        stop=(kt == KT - 1),
                    )
                nc.vector.tensor_copy(
                    vsb[:, i * HPC * VE : (i + 1) * HPC * VE].rearrange(
                        "p (h e) -> p h e", e=VE
                    )[:, :, 0:DH],
                    vps.rearrange("p (h d) -> p h d", d=DH),
                )

            # Q/K projections, one [128, 512] tile at a time (emitted as PE
            # filler work inside earlier attention loops).
            qk_tiles = {}

            def qk_tile(p, wsel, jq):
                if p not in qk_tiles:
                    qt2 = qkpool.tile([128, S], bf16, tag="qt2", name="qt2")
                    kt2 = qkpool.tile([128, S], bf16, tag="kt2", name="kt2")
                    qk_tiles[p] = (qt2, kt2)
                wt = (wq_t, wk_t)[wsel]
                dst = qk_tiles[p][wsel]
                pps = ps.tile([128, 512], f32, tag="mm512", bufs=2, name="pps")
                for kt in range(KT):
                    nc.tensor.matmul(
                        pps[:],
                        wt[:, kt * HPC * DH + p * 128 : kt * HPC * DH + (p + 1) * 128],
                        et_t[:, kt * S + jq * F : kt * S + (jq + 1) * F],
                        start=(kt == 0),
                        stop=(kt == KT - 1),
                    )
                nc.vector.tensor_copy(dst[:, jq * F : (jq + 1) * F], pps[:])

            headsT = [
                hpool.tile([128, S], bf16, name=f"headsT{t}", tag=f"headsT{t}")
                for t in range(PAIRS)
            ]

            # Deferred work (emitted one qb later to hide DVE latency
            # behind the next qb's matmul stream).
            pending = []

            def flush_pending():
                while pending:
                    pending.pop(0)()

            def make_tail(p, qb, normed, jps=(0, 1)):
                """Transposes (and for the last pair, w_o) for (p, qb)."""

                def emit():
                    # Late in the last pair the Activation engine has drained
                    # its exp queue; route evacuations there to unload DVE.
                    on_act = False
                    copy = nc.scalar.copy if on_act else nc.vector.tensor_copy
                    for jp in jps:
                        for j in (2 * jp, 2 * jp + 1):
                            st = qb * NQB + j
                            tps = ps.tile([128, 512], f32, tag="mm512", bufs=2, name="tps")
                            tps_bf = tps.bitcast(bf16)
                            nc.tensor.matmul(
                                tps_bf[:, 0:CH],
                                normed[:, j * CH : (j + 1) * CH],
                                id_t[:],
                                is_transpose=True,
                            )
                            with tc.high_priority(offset=300):
                                copy(
                                    headsT[p][:, st * CH : (st + 1) * CH], tps_bf[:, 0:CH]
                                )
                            if p == PAIRS - 1:
                                ot = opool.tile([128, DM], f32, tag="ot", name="ot")
                                for nh in range(2):
                                    wps = ps.tile([128, 512], f32, tag="mm512", bufs=2, name="wps")
                                    for ktt in range(KO):
                                        nc.tensor.matmul(
                                            wps[:],
                                            headsT[ktt][:, st * CH : (st + 1) * CH],
                                            wo_t[:, ktt * DM + nh * 512 : ktt * DM + (nh + 1) * 512],
                                            start=(ktt == 0),
                                            stop=(ktt == KO - 1),
                                        )
                                    copy(ot[:, nh * 512 : (nh + 1) * 512], wps[:])
                                    nc.sync.dma_start(
                                        out[st * CH : (st + 1) * CH, nh * 512 : (nh + 1) * 512],
                                        ot[:, nh * 512 : (nh + 1) * 512],
                                    )

                return emit

            # PE filler for slot (p, qb): projections needed strictly later.
            def fillers(p, qb):
                if p == 0 and qb < 3:
                    for i in range(4 * qb + 4, 4 * qb + 8):
                        v_chunk(i)
                if qb < 3:
                    qk_tile(p, 0, qb + 1)
                    qk_tile(p, 1, qb + 1)
                elif p + 1 < PAIRS:
                    qk_tile(p + 1, 0, 0)
                    qk_tile(p + 1, 1, 0)

            # lead-in: V chunks + first pair's first q/k tiles
            for i in range(4):
                v_chunk(i)
            qk_tile(0, 0, 0)
            qk_tile(0, 1, 0)

            for p in range(PAIRS):
                qt2, kt2 = qk_tiles[p]
                for qb in range(NQB):
                    nch = 4 * qb + 4 if causal else NT
                    e_grp = epool.tile([128, NT * 2 * F], bf16, tag="e", name="e_grp")

                    # r0: first causally-live query column within this qb
                    # block for chunk c (block-granular band narrowing)
                    def _r0(c):
                        return (c - 4 * qb) * CH if causal and c >= 4 * qb else 0

                    # psum banks for PV, zeroed while logits run
                    pv_t = [
                        ps.tile([128, 512], f32, tag="pv", bufs=2, name="pv")
                        for _ in range(2)
                    ]

                    # ---- logits + exp (+ diagonal band mask on Pool) ----
                    for c in range(nch):
                        r0 = _r0(c)
                        stg = ps.tile([128, 2 * F], f32, tag="stg", bufs=2, name="stg")
                        for hh in (0, 1):
                            nc.tensor.matmul(
                                stg[:, hh * F + r0 : (hh + 1) * F],
                                kt2[64 * hh : 64 * hh + 64, c * CH : (c + 1) * CH],
                                qt2[64 * hh : 64 * hh + 64, qb * F + r0 : (qb + 1) * F],
                                start=True,
                                stop=True,
                            )
                        st3 = stg.rearrange("p (h f) -> p h f", h=2)[:, :, r0:F]
                        ex3 = e_grp[:, 2 * c * F : (2 * c + 2) * F].rearrange(
                            "p (h f) -> p h f", h=2
                        )[:, :, r0:F]
                        nc.scalar.activation(ex3, st3, Exp, scale=SCALE)
                        if causal and c >= 4 * qb:
                            # staircase mask on the diagonal 128-col block
                            j = c - 4 * qb
                            sl = e_grp[:, 2 * c * F : (2 * c + 2) * F].rearrange(
                                "p (h f) -> p h f", h=2
                            )[:, :, j * CH : (j + 1) * CH]
                            nc.vector.tensor_mul(
                                sl, sl, band_t.rearrange("p (h f) -> p h f", h=2)
                            )

                    fillers(p, qb)

                    # ---- PV (transposed): out[q, (j2,hh,65)] per j-pair ----
                    # Four accumulation groups (2 qtiles x 2 heads) share each
                    # psum bank.  A matmul with start=True would zero the
                    # whole 2KB region (clobbering sibling groups), so the
                    # bank was zeroed by the memset above and every matmul
                    # accumulates (start=False).
                    for jp in range(2):
                        jlo, jhi = 2 * jp, 2 * jp + 1
                        pv = pv_t[jp]
                        clast = (4 * qb + jhi) if causal else NT - 1
                        for c in range(clast + 1):
                            for jloc, j in ((0, jlo), (1, jhi)):
                                cg_last = (4 * qb + j) if causal else NT - 1
                                if c > cg_last:
                                    continue
                                for hh in (0, 1):
                                    nc.tensor.matmul(
                                        pv[:, (jloc * 2 + hh) * VE : (jloc * 2 + hh + 1) * VE],
                                        e_grp[:, (2 * c + hh) * F + j * CH : (2 * c + hh) * F + (j + 1) * CH],
                                        vsb[:, c * HPC * VE + (2 * p + hh) * VE : c * HPC * VE + (2 * p + hh + 1) * VE],
                                        start=(c == 0 and jloc == 0 and hh == 0),
                                        stop=(c == cg_last),
                                        skip_group_check=True,
                                    )
                        if jp == 0:
                            normed = npool.tile([128, 512], bf16, tag="normed", name="normed")
                        recip4 = npool.tile([128, 4], f32, tag="recip4", name="recip4")
                        pv4 = pv[:, 0 : 4 * VE].rearrange("p (j h e) -> p j h e", j=2, h=2)
                        with tc.high_priority(offset=400):
                            nc.vector.reciprocal(
                                recip4.rearrange("p (j h o) -> p j h o", j=2, h=2),
                                pv4[:, :, :, DH : DH + 1],
                            )
                            nc.vector.tensor_mul(
                                normed[:, jp * 256 : (jp + 1) * 256].rearrange(
                                    "p (j h e) -> p j h e", j=2, h=2
                                ),
                                pv4[:, :, :, 0:DH],
                                recip4.rearrange("p (j h o) -> p j h o", j=2, h=2)
                                .broadcast_to([128, 2, 2, DH]),
                            )

                    # ---- deferred transposes / w_o from the previous qb ----
                    flush_pending()
                    if p == PAIRS - 1 and qb == NQB - 1:
                        # final tail: emit inline, split by j-pair so the
                        # first transposes overlap the last PV/norm chain
                        make_tail(p, qb, normed, jps=(0,))()
                        make_tail(p, qb, normed, jps=(1,))()
                    else:
                        pending.append(make_tail(p, qb, normed))

            flush_pending()

    _split_excess_waits(nc)
    return nc


# revision 5
# speedup vs baseline: 1.0318x; 1.0033x over previous
"""Multi-head attention (B=4, S=2048, DM=1024, H=16, DH=64) on 8 TRN2 cores.

Sharding: 8 cores = 4 batches x 2 head-halves. Core c handles batch c//2 and
heads [ (c%2)*8, (c%2)*8+8 ).  Each core projects Q/K/V for its 8 heads,
runs causal softmax attention, applies its slice of w_o, and writes a
partial [S, DM] output.  The host sums the two partials per batch.

Attention uses a transposed-PV formulation: exp(logits) is kept in
[kv, q] layout and used as the *stationary* operand of the PV matmuls,
producing head outputs in [q, (head, d)] layout with a fused ones-column
denominator.  After a per-partition reciprocal multiply the [q, d] tiles
are transposed back to [d, q] on the PE (identity-matmul transpose) to
feed w_o.  All matmuls are bf16 with fp32 PSUM accumulation.
"""

import math

import ml_dtypes
import numpy as np

B, S, DM, H, DH = 4, 2048, 1024, 16, 64
NCORES = 8
HPC = H // 2        # heads per core
PAIRS = HPC // 2    # head pairs per core (packed 2-per-128-partitions)
F = 512             # query block (free dim of QK matmuls)
CH = 128            # kv chunk (partition dim of transposed logits)
NQB = S // F        # query blocks
NT = S // CH        # kv chunks
VE = DH + 1         # V extended with a ones column (fused denominator)
KT = DM // 128      # contraction k-tiles for projections
KO = HPC * DH // 128  # contraction k-tiles for w_o
SCALE = 1.0 / math.sqrt(DH)

_CACHE = {}


def _split_excess_waits(nc):
    """This environment's walrus rejects instructions carrying more than one
    sync wait ("Too many sync wait commands").  Hoist excess waits onto
    single-wait NoOps inserted right before the offending instruction."""
    import concourse.mybir as mybir

    n = 0
    for f in nc.m.functions:
        for blk in f.blocks:
            newlist = []
            for ins in blk.instructions:
                si = ins.sync_info
                if si is not None and len(si.on_wait) > 1:
                    for w in si.on_wait[:-1]:
                        n += 1
                        newlist.append(
                            mybir.InstNoOp(
                                name=f"I-waitfix-{n}",
                                opcode="NoOp",
                                engine=ins.engine,
                                sync_info=mybir.SyncInfo(on_wait=[w], on_update=[]),
                            )
                        )
                    si.on_wait = si.on_wait[-1:]
                newlist.append(ins)
            blk.instructions = newlist
    return n


def _build(causal):
    import concourse.bass as bass
    import concourse.mybir as mybir
    import concourse.tile as tile

    bf16 = mybir.dt.bfloat16
    f32 = mybir.dt.float32
    Exp = mybir.ActivationFunctionType.Exp

    nc = bass.Bass()
    et = nc.dram_tensor("et", [DM, S], bf16, kind="ExternalInput")
    wq = nc.dram_tensor("wq", [DM, HPC * DH], bf16, kind="ExternalInput")
    wk = nc.dram_tensor("wk", [DM, HPC * DH], bf16, kind="ExternalInput")
    wv = nc.dram_tensor("wv", [DM, HPC * DH], bf16, kind="ExternalInput")
    wo = nc.dram_tensor("wo", [HPC * DH, DM], bf16, kind="ExternalInput")
    band2 = nc.dram_tensor("band2", [CH, 2 * CH], bf16, kind="ExternalInput")
    ident = nc.dram_tensor("ident", [CH, CH], bf16, kind="ExternalInput")
    out = nc.dram_tensor("out", [S, DM], f32, kind="ExternalOutput")

    with tile.TileContext(nc) as tc:
        with tc.tile_pool(name="const", bufs=1) as cpool, \
             tc.tile_pool(name="qk", bufs=2) as qkpool, \
             tc.tile_pool(name="eexp", bufs=2) as epool, \
             tc.tile_pool(name="heads", bufs=1) as hpool, \
             tc.tile_pool(name="norm", bufs=2) as npool, \
             tc.tile_pool(name="outp", bufs=2) as opool, \
             tc.tile_pool(name="ps", bufs=1, space="PSUM") as ps:

            # ---- input loads.  All DMA transfers serialize on the shared
            # DMA engine pool, so order them so the first V-projection
            # matmul (needs wv kt0 + et chunk-0 kt0) can start ~2us in. ----
            wv_t = cpool.tile([128, KT * HPC * DH], bf16, name="wv_t")
            et_t = cpool.tile([128, KT * S], bf16, name="et_t")
            etr = et.rearrange("(a p) n -> p a n", p=128)
            ett = et_t.rearrange("p (a n) -> p a n", a=KT)
            NQ4 = S // 4
            nc.sync.dma_start(ett[:, :, 0:NQ4], etr[:, :, 0:NQ4])
            for kt in range(KT):
                nc.scalar.dma_start(
                    wv_t[:, kt * HPC * DH : (kt + 1) * HPC * DH],
                    wv[kt * 128 : (kt + 1) * 128, :],
                )
            nc.sync.dma_start(ett[:, :, NQ4 : 2 * NQ4], etr[:, :, NQ4 : 2 * NQ4])
            # weights for the first attention pair; issued on the Act queue
            # AFTER the wv pieces so their DMA-engine service slots land
            # between et chunks (service is FIFO by arrival time).
            w_tiles = {}
            for nm, src in (("wq", wq), ("wk", wk)):
                t = cpool.tile([128, KT * HPC * DH], bf16, name=f"{nm}_t")
                nc.scalar.dma_start(
                    t.rearrange("p (a n) -> p a n", a=KT),
                    src.rearrange("(a p) n -> p a n", p=128),
                )
                w_tiles[nm] = t
            wq_t, wk_t = w_tiles["wq"], w_tiles["wk"]
            for cq in range(2, 4):
                nc.scalar.dma_start(
                    ett[:, :, cq * NQ4 : (cq + 1) * NQ4],
                    etr[:, :, cq * NQ4 : (cq + 1) * NQ4],
                )
            wo_t = cpool.tile([128, KO * DM], bf16, name="wo_t")
            nc.scalar.dma_start(
                wo_t.rearrange("p (a n) -> p a n", a=KO),
                wo.rearrange("(a p) n -> p a n", p=128),
            )
            band_t = cpool.tile([CH, 2 * CH], bf16, name="band_t")
            nc.sync.dma_start(band_t[:], band2[:])
            id_t = cpool.tile([CH, CH], bf16, name="id_t")
            nc.sync.dma_start(id_t[:], ident[:])

            # ---- V projection: vsb per kv chunk i: [128, 8x(64+ones)] ----
            vsb = cpool.tile([128, NT * HPC * VE], bf16, name="vsb")
            nc.vector.memset(
                vsb.rearrange("p (i e) -> p i e", e=VE)[:, :, DH:VE], 1.0
            )

            # PE p-state warmup: the tensor engine reaches full clock only
            # after ~3us of continuous busy.  Burn the ramp on throwaway
            # matmuls over a memset tile (no DMA dependency) while the
            # first input DMAs stream in, so real work starts warm.
            wrm = cpool.tile([128, 512], bf16, name="wrm")
            nc.vector.memset(wrm[:], 1.0)
            for _ in range(24):
                wps0 = ps.tile([128, 512], f32, tag="mm512", bufs=2, name="wps0")
                nc.tensor.matmul(
                    wps0[:], wrm[:, 0:128], wrm[:], start=True, stop=True
                )

            def v_chunk(i):
                vps = ps.tile([128, 512], f32, tag="mm512", bufs=2, name="vps")
                for kt in range(KT):
                    nc.tensor.matmul(
                        vps[:],
                        et_t[:, kt * S + i * CH : kt * S + (i + 1) * CH],
                        wv_t[:, kt * HPC * DH : (kt + 1) * HPC * DH],
                        start=(kt == 0),
                        stop=(kt == KT - 1),
                    )
                nc.vector.tensor_copy(
                    vsb[:, i * HPC * VE : (i + 1) * HPC * VE].rearrange(
                        "p (h e) -> p h e", e=VE
                    )[:, :, 0:DH],
                    vps.rearrange("p (h d) -> p h d", d=DH),
                )

            # Q/K projections, one [128, 512] tile at a time (emitted as PE
            # filler work inside earlier attention loops).
            qk_tiles = {}

            def qk_tile(p, wsel, jq):
                if p not in qk_tiles:
                    qt2 = qkpool.tile([128, S], bf16, tag="qt2", name="qt2")
                    kt2 = qkpool.tile([128, S], bf16, tag="kt2", name="kt2")
                    qk_tiles[p] = (qt2, kt2)
                wt = (wq_t, wk_t)[wsel]
                dst = qk_tiles[p][wsel]
                pps = ps.tile([128, 512], f32, tag="mm512", bufs=2, name="pps")
                for kt in range(KT):
                    nc.tensor.matmul(
                        pps[:],
                        wt[:, kt * HPC * DH + p * 128 : kt * HPC * DH + (p + 1) * 128],
                        et_t[:, kt * S + jq * F : kt * S + (jq + 1) * F],
                        start=(kt == 0),
                        stop=(kt == KT - 1),
                    )
                nc.vector.tensor_copy(dst[:, jq * F : (jq + 1) * F], pps[:])

            headsT = [
                hpool.tile([128, S], bf16, name=f"headsT{t}", tag=f"headsT{t}")
                for t in range(PAIRS)
            ]

            # Deferred work (emitted one qb later to hide DVE latency
            # behind the next qb's matmul stream).
            pending = []

            def flush_pending():
                while pending:
                    pending.pop(0)()

            def make_tail(p, qb, normed, jps=(0, 1)):
                """Transposes (and for the last pair, w_o) for (p, qb)."""

                def emit():
                    # Late in the last pair the Activation engine has drained
                    # its exp queue; route evacuations there to unload DVE.
                    on_act = False
                    copy = nc.scalar.copy if on_act else nc.vector.tensor_copy
                    for jp in jps:
                        for j in (2 * jp, 2 * jp + 1):
                            st = qb * NQB + j
                            tps = ps.tile([128, 512], f32, tag="mm512", bufs=2, name="tps")
                            tps_bf = tps.bitcast(bf16)
                            nc.tensor.matmul(
                                tps_bf[:, 0:CH],
                                normed[:, j * CH : (j + 1) * CH],
                                id_t[:],
                                is_transpose=True,
                            )
                            with tc.high_priority(offset=300):
                                copy(
                                    headsT[p][:, st * CH : (st + 1) * CH], tps_bf[:, 0:CH]
                                )
                            if p == PAIRS - 1:
                                ot = opool.tile([128, DM], f32, tag="ot", name="ot")
                                for nh in range(2):
                                    wps = ps.tile([128, 512], f32, tag="mm512", bufs=2, name="wps")
                                    for ktt in range(KO):
                                        nc.tensor.matmul(
                                            wps[:],
                                            headsT[ktt][:, st * CH : (st + 1) * CH],
                                            wo_t[:, ktt * DM + nh * 512 : ktt * DM + (nh + 1) * 512],
                                            start=(ktt == 0),
                                            stop=(ktt == KO - 1),
                                        )
                                    copy(ot[:, nh * 512 : (nh + 1) * 512], wps[:])
                                    nc.sync.dma_start(
                                        out[st * CH : (st + 1) * CH, nh * 512 : (nh + 1) * 512],
                                        ot[:, nh * 512 : (nh + 1) * 512],
                                    )

                return emit

            # PE filler for slot (p, qb): projections needed strictly later.
            def fillers(p, qb):
                if p == 0 and qb < 3:
                    for i in range(4 * qb + 4, 4 * qb + 8):
                        v_chunk(i)
                if qb < 3:
                    qk_tile(p, 0, qb + 1)
                    qk_tile(p, 1, qb + 1)
                elif p + 1 < PAIRS:
                    qk_tile(p + 1, 0, 0)
                    qk_tile(p + 1, 1, 0)

            # lead-in: V chunks + first pair's first q/k tiles
            for i in range(4):
                v_chunk(i)
            qk_tile(0, 0, 0)
            qk_tile(0, 1, 0)

            for p in range(PAIRS):
                qt2, kt2 = qk_tiles[p]
                for qb in range(NQB):
                    nch = 4 * qb + 4 if causal else NT
                    e_grp = epool.tile([128, NT * 2 * F], bf16, tag="e", name="e_grp")

                    # r0: first causally-live query column within this qb
                    # block for chunk c (block-granular band narrowing)
                    def _r0(c):
                        return (c - 4 * qb) * CH if causal and c >= 4 * qb else 0

                    # psum banks for PV, zeroed while logits run
                    pv_t = [
                        ps.tile([128, 512], f32, tag="pv", bufs=2, name="pv")
                        for _ in range(2)
                    ]

                    # ---- logits + exp (+ diagonal band mask on Pool) ----
                    for c in range(nch):
                        r0 = _r0(c)
                        stg = ps.tile([128, 2 * F], f32, tag="stg", bufs=2, name="stg")
                        for hh in (0, 1):
                            nc.tensor.matmul(
                                stg[:, hh * F + r0 : (hh + 1) * F],
                                kt2[64 * hh : 64 * hh + 64, c * CH : (c + 1) * CH],
                                qt2[64 * hh : 64 * hh + 64, qb * F + r0 : (qb + 1) * F],
                                start=True,
                                stop=True,
                            )
                        st3 = stg.rearrange("p (h f) -> p h f", h=2)[:, :, r0:F]
                        ex3 = e_grp[:, 2 * c * F : (2 * c + 2) * F].rearrange(
                            "p (h f) -> p h f", h=2
                        )[:, :, r0:F]
                        nc.scalar.activation(ex3, st3, Exp, scale=SCALE)
                        if causal and c >= 4 * qb:
                            # staircase mask on the diagonal 128-col block
                            j = c - 4 * qb
                            sl = e_grp[:, 2 * c * F : (2 * c + 2) * F].rearrange(
                                "p (h f) -> p h f", h=2
                            )[:, :, j * CH : (j + 1) * CH]
                            nc.vector.tensor_mul(
                                sl, sl, band_t.rearrange("p (h f) -> p h f", h=2)
                            )

                    fillers(p, qb)

                    # ---- PV (transposed): out[q, (j2,hh,65)] per j-pair ----
                    # Four accumulation groups (2 qtiles x 2 heads) share each
                    # psum bank.  A matmul with start=True would zero the
                    # whole 2KB region (clobbering sibling groups), so the
                    # bank was zeroed by the memset above and every matmul
                    # accumulates (start=False).
                    for jp in range(2):
                        jlo, jhi = 2 * jp, 2 * jp + 1
                        pv = pv_t[jp]
                        clast = (4 * qb + jhi) if causal else NT - 1
                        for c in range(clast + 1):
                            for jloc, j in ((0, jlo), (1, jhi)):
                                cg_last = (4 * qb + j) if causal else NT - 1
                                if c > cg_last:
                                    continue
                                for hh in (0, 1):
                                    nc.tensor.matmul(
                                        pv[:, (jloc * 2 + hh) * VE : (jloc * 2 + hh + 1) * VE],
                                        e_grp[:, (2 * c + hh) * F + j * CH : (2 * c + hh) * F + (j + 1) * CH],
                                        vsb[:, c * HPC * VE + (2 * p + hh) * VE : c * HPC * VE + (2 * p + hh + 1) * VE],
                                        start=(c == 0 and jloc == 0 and hh == 0),
                                        stop=(c == cg_last),
                                        skip_group_check=True,
                                    )
                        if jp == 0:
                            normed = npool.tile([128, 512], bf16, tag="normed", name="normed")
                        recip4 = npool.tile([128, 4], f32, tag="recip4", name="recip4")
                        pv4 = pv[:, 0 : 4 * VE].rearrange("p (j h e) -> p j h e", j=2, h=2)
                        with tc.high_priority(offset=400):
                            nc.vector.reciprocal(
                                recip4.rearrange("p (j h o) -> p j h o", j=2, h=2),
                                pv4[:, :, :, DH : DH + 1],
                            )
                            nc.vector.tensor_mul(
                                normed[:, jp * 256 : (jp + 1) * 256].rearrange(
                                    "p (j h e) -> p j h e", j=2, h=2
                                ),
                                pv4[:, :, :, 0:DH],
                                recip4.rearrange("p (j h o) -> p j h o", j=2, h=2)
                                .broadcast_to([128, 2, 2, DH]),
                            )

                    # ---- deferred transposes / w_o from the previous qb ----
                    flush_pending()
                    if p == PAIRS - 1 and qb == NQB - 1:
                        # final tail: emit inline, split by j-pair so the
                        # first transposes overlap the last PV/norm chain
                        make_tail(p, qb, normed, jps=(0,))()
                        make_tail(p, qb, normed, jps=(1,))()
                    else:
                        pending.append(make_tail(p, qb, normed))

            flush_pending()

    _split_excess_waits(nc)
    return nc


# revision 6
# speedup vs baseline: 1.0346x; 1.0027x over previous
"""Multi-head attention (B=4, S=2048, DM=1024, H=16, DH=64) on 8 TRN2 cores.

Sharding: 8 cores = 4 batches x 2 head-halves. Core c handles batch c//2 and
heads [ (c%2)*8, (c%2)*8+8 ).  Each core projects Q/K/V for its 8 heads,
runs causal softmax attention, applies its slice of w_o, and writes a
partial [S, DM] output.  The host sums the two partials per batch.

Attention uses a transposed-PV formulation: exp(logits) is kept in
[kv, q] layout and used as the *stationary* operand of the PV matmuls,
producing head outputs in [q, (head, d)] layout with a fused ones-column
denominator.  After a per-partition reciprocal multiply the [q, d] tiles
are transposed back to [d, q] on the PE (identity-matmul transpose) to
feed w_o.  All matmuls are bf16 with fp32 PSUM accumulation.
"""

import math

import ml_dtypes
import numpy as np

B, S, DM, H, DH = 4, 2048, 1024, 16, 64
NCORES = 8
HPC = H // 2        # heads per core
PAIRS = HPC // 2    # head pairs per core (packed 2-per-128-partitions)
F = 512             # query block (free dim of QK matmuls)
CH = 128            # kv chunk (partition dim of transposed logits)
NQB = S // F        # query blocks
NT = S // CH        # kv chunks
VE = DH + 1         # V extended with a ones column (fused denominator)
KT = DM // 128      # contraction k-tiles for projections
KO = HPC * DH // 128  # contraction k-tiles for w_o
SCALE = 1.0 / math.sqrt(DH)

_CACHE = {}


def _split_excess_waits(nc):
    """This environment's walrus rejects instructions carrying more than one
    sync wait ("Too many sync wait commands").  Hoist excess waits onto
    single-wait NoOps inserted right before the offending instruction."""
    import concourse.mybir as mybir

    n = 0
    for f in nc.m.functions:
        for blk in f.blocks:
            newlist = []
            for ins in blk.instructions:
                si = ins.sync_info
                if si is not None and len(si.on_wait) > 1:
                    for w in si.on_wait[:-1]:
                        n += 1
                        newlist.append(
                            mybir.InstNoOp(
                                name=f"I-waitfix-{n}",
                                opcode="NoOp",
                                engine=ins.engine,
                                sync_info=mybir.SyncInfo(on_wait=[w], on_update=[]),
                            )
                        )
                    si.on_wait = si.on_wait[-1:]
                newlist.append(ins)
            blk.instructions = newlist
    return n


def _build(causal):
    import concourse.bass as bass
    import concourse.mybir as mybir
    import concourse.tile as tile

    bf16 = mybir.dt.bfloat16
    f32 = mybir.dt.float32
    Exp = mybir.ActivationFunctionType.Exp

    nc = bass.Bass()
    et = nc.dram_tensor("et", [DM, S], bf16, kind="ExternalInput")
    wq = nc.dram_tensor("wq", [DM, HPC * DH], bf16, kind="ExternalInput")
    wk = nc.dram_tensor("wk", [DM, HPC * DH], bf16, kind="ExternalInput")
    wv = nc.dram_tensor("wv", [DM, HPC * DH], bf16, kind="ExternalInput")
    wo = nc.dram_tensor("wo", [HPC * DH, DM], bf16, kind="ExternalInput")
    band2 = nc.dram_tensor("band2", [CH, 2 * CH], bf16, kind="ExternalInput")
    ident = nc.dram_tensor("ident", [CH, CH], bf16, kind="ExternalInput")
    out = nc.dram_tensor("out", [S, DM], f32, kind="ExternalOutput")

    with tile.TileContext(nc) as tc:
        with tc.tile_pool(name="const", bufs=1) as cpool, \
             tc.tile_pool(name="qk", bufs=2) as qkpool, \
             tc.tile_pool(name="eexp", bufs=2) as epool, \
             tc.tile_pool(name="heads", bufs=1) as hpool, \
             tc.tile_pool(name="norm", bufs=2) as npool, \
             tc.tile_pool(name="outp", bufs=2) as opool, \
             tc.tile_pool(name="ps", bufs=1, space="PSUM") as ps:

            # ---- input loads.  All DMA transfers serialize on the shared
            # DMA engine pool, so order them so the first V-projection
            # matmul (needs wv kt0 + et chunk-0 kt0) can start ~2us in. ----
            wv_t = cpool.tile([128, KT * HPC * DH], bf16, name="wv_t")
            et_t = cpool.tile([128, KT * S], bf16, name="et_t")
            etr = et.rearrange("(a p) n -> p a n", p=128)
            ett = et_t.rearrange("p (a n) -> p a n", a=KT)
            NQ4 = S // 4
            nc.sync.dma_start(ett[:, :, 0:NQ4], etr[:, :, 0:NQ4])
            for kt in range(KT):
                nc.scalar.dma_start(
                    wv_t[:, kt * HPC * DH : (kt + 1) * HPC * DH],
                    wv[kt * 128 : (kt + 1) * 128, :],
                )
            nc.sync.dma_start(ett[:, :, NQ4 : 2 * NQ4], etr[:, :, NQ4 : 2 * NQ4])
            # weights for the first attention pair; issued on the Act queue
            # AFTER the wv pieces so their DMA-engine service slots land
            # between et chunks (service is FIFO by arrival time).
            w_tiles = {}
            for nm, src in (("wq", wq), ("wk", wk)):
                t = cpool.tile([128, KT * HPC * DH], bf16, name=f"{nm}_t")
                nc.scalar.dma_start(
                    t.rearrange("p (a n) -> p a n", a=KT),
                    src.rearrange("(a p) n -> p a n", p=128),
                )
                w_tiles[nm] = t
            wq_t, wk_t = w_tiles["wq"], w_tiles["wk"]
            for cq in range(2, 4):
                nc.scalar.dma_start(
                    ett[:, :, cq * NQ4 : (cq + 1) * NQ4],
                    etr[:, :, cq * NQ4 : (cq + 1) * NQ4],
                )
            wo_t = cpool.tile([128, KO * DM], bf16, name="wo_t")
            nc.scalar.dma_start(
                wo_t.rearrange("p (a n) -> p a n", a=KO),
                wo.rearrange("(a p) n -> p a n", p=128),
            )
            band_t = cpool.tile([CH, 2 * CH], bf16, name="band_t")
            nc.sync.dma_start(band_t[:], band2[:])
            id_t = cpool.tile([CH, CH], bf16, name="id_t")
            nc.sync.dma_start(id_t[:], ident[:])

            # ---- V projection: vsb per kv chunk i: [128, 8x(64+ones)] ----
            vsb = cpool.tile([128, NT * HPC * VE], bf16, name="vsb")
            nc.vector.memset(
                vsb.rearrange("p (i e) -> p i e", e=VE)[:, :, DH:VE], 1.0
            )

            # PE p-state warmup: the tensor engine reaches full clock only
            # after ~3us of continuous busy.  Burn the ramp on throwaway
            # matmuls over a memset tile (no DMA dependency) while the
            # first input DMAs stream in, so real work starts warm.
            wrm = cpool.tile([128, 512], bf16, name="wrm")
            nc.vector.memset(wrm[:], 1.0)
            for _ in range(24):
                wps0 = ps.tile([128, 512], f32, tag="mm512", bufs=2, name="wps0")
                nc.tensor.matmul(
                    wps0[:], wrm[:, 0:128], wrm[:], start=True, stop=True
                )

            def v_chunk(i):
                vps = ps.tile([128, 512], f32, tag="mm512", bufs=2, name="vps")
                for kt in range(KT):
                    nc.tensor.matmul(
                        vps[:],
                        et_t[:, kt * S + i * CH : kt * S + (i + 1) * CH],
                        wv_t[:, kt * HPC * DH : (kt + 1) * HPC * DH],
                        start=(kt == 0),
                        stop=(kt == KT - 1),
                    )
                nc.vector.tensor_copy(
                    vsb[:, i * HPC * VE : (i + 1) * HPC * VE].rearrange(
                        "p (h e) -> p h e", e=VE
                    )[:, :, 0:DH],
                    vps.rearrange("p (h d) -> p h d", d=DH),
                )

            # Q/K projections, one [128, 512] tile at a time (emitted as PE
            # filler work inside earlier attention loops).
            qk_tiles = {}

            def qk_tile(p, wsel, jq):
                if p not in qk_tiles:
                    qt2 = qkpool.tile([128, S], bf16, tag="qt2", name="qt2")
                    kt2 = qkpool.tile([128, S], bf16, tag="kt2", name="kt2")
                    qk_tiles[p] = (qt2, kt2)
                wt = (wq_t, wk_t)[wsel]
                dst = qk_tiles[p][wsel]
                pps = ps.tile([128, 512], f32, tag="mm512", bufs=2, name="pps")
                for kt in range(KT):
                    nc.tensor.matmul(
                        pps[:],
                        wt[:, kt * HPC * DH + p * 128 : kt * HPC * DH + (p + 1) * 128],
                        et_t[:, kt * S + jq * F : kt * S + (jq + 1) * F],
                        start=(kt == 0),
                        stop=(kt == KT - 1),
                    )
                nc.vector.tensor_copy(dst[:, jq * F : (jq + 1) * F], pps[:])

            headsT = [
                hpool.tile([128, S], bf16, name=f"headsT{t}", tag=f"headsT{t}")
                for t in range(PAIRS)
            ]

            # Deferred work (emitted one qb later to hide DVE latency
            # behind the next qb's matmul stream).
            pending = []

            def flush_pending():
                while pending:
                    pending.pop(0)()

            def make_tail(p, qb, normed, jps=(0, 1)):
                """Transposes (and for the last pair, w_o) for (p, qb)."""

                def emit():
                    copy = nc.vector.tensor_copy
                    for jp in jps:
                        for j in (2 * jp, 2 * jp + 1):
                            st = qb * NQB + j
                            tps = ps.tile([128, 512], f32, tag="mm512", bufs=2, name="tps")
                            tps_bf = tps.bitcast(bf16)
                            nc.tensor.matmul(
                                tps_bf[:, 0:CH],
                                normed[:, j * CH : (j + 1) * CH],
                                id_t[:],
                                is_transpose=True,
                            )
                            with tc.high_priority(offset=300):
                                copy(
                                    headsT[p][:, st * CH : (st + 1) * CH], tps_bf[:, 0:CH]
                                )
                    for jp in jps:
                        for j in (2 * jp, 2 * jp + 1):
                            st = qb * NQB + j
                            if p == PAIRS - 1:
                                ot = opool.tile([128, DM], f32, tag="ot", name="ot")
                                for nh in range(2):
                                    wps = ps.tile([128, 512], f32, tag="mm512", bufs=2, name="wps")
                                    for ktt in range(KO):
                                        nc.tensor.matmul(
                                            wps[:],
                                            headsT[ktt][:, st * CH : (st + 1) * CH],
                                            wo_t[:, ktt * DM + nh * 512 : ktt * DM + (nh + 1) * 512],
                                            start=(ktt == 0),
                                            stop=(ktt == KO - 1),
                                        )
                                    copy(ot[:, nh * 512 : (nh + 1) * 512], wps[:])
                                    nc.sync.dma_start(
                                        out[st * CH : (st + 1) * CH, nh * 512 : (nh + 1) * 512],
                                        ot[:, nh * 512 : (nh + 1) * 512],
                                    )

                return emit

            # PE filler for slot (p, qb): projections needed strictly later.
            def fillers(p, qb):
                if p == 0 and qb < 3:
                    for i in range(4 * qb + 4, 4 * qb + 8):
                        v_chunk(i)
                if qb < 3:
                    qk_tile(p, 0, qb + 1)
                    qk_tile(p, 1, qb + 1)
                elif p + 1 < PAIRS:
                    qk_tile(p + 1, 0, 0)
                    qk_tile(p + 1, 1, 0)

            # lead-in: V chunks + first pair's first q/k tiles
            for i in range(4):
                v_chunk(i)
            qk_tile(0, 0, 0)
            qk_tile(0, 1, 0)

            for p in range(PAIRS):
                qt2, kt2 = qk_tiles[p]
                for qb in range(NQB):
                    nch = 4 * qb + 4 if causal else NT
                    e_grp = epool.tile([128, NT * 2 * F], bf16, tag="e", name="e_grp")

                    # r0: first causally-live query column within this qb
                    # block for chunk c (block-granular band narrowing)
                    def _r0(c):
                        return (c - 4 * qb) * CH if causal and c >= 4 * qb else 0

                    # psum banks for PV, zeroed while logits run
                    pv_t = [
                        ps.tile([128, 512], f32, tag="pv", bufs=2, name="pv")
                        for _ in range(2)
                    ]

                    # ---- logits + exp (+ diagonal band mask on Pool) ----
                    for c in range(nch):
                        r0 = _r0(c)
                        stg = ps.tile([128, 2 * F], f32, tag="stg", bufs=2, name="stg")
                        for hh in (0, 1):
                            nc.tensor.matmul(
                                stg[:, hh * F + r0 : (hh + 1) * F],
                                kt2[64 * hh : 64 * hh + 64, c * CH : (c + 1) * CH],
                                qt2[64 * hh : 64 * hh + 64, qb * F + r0 : (qb + 1) * F],
                                start=True,
                                stop=True,
                            )
                        st3 = stg.rearrange("p (h f) -> p h f", h=2)[:, :, r0:F]
                        ex3 = e_grp[:, 2 * c * F : (2 * c + 2) * F].rearrange(
                            "p (h f) -> p h f", h=2
                        )[:, :, r0:F]
                        nc.scalar.activation(ex3, st3, Exp, scale=SCALE)
                        if causal and c >= 4 * qb:
                            # staircase mask on the diagonal 128-col block
                            j = c - 4 * qb
                            sl = e_grp[:, 2 * c * F : (2 * c + 2) * F].rearrange(
                                "p (h f) -> p h f", h=2
                            )[:, :, j * CH : (j + 1) * CH]
                            nc.vector.tensor_mul(
                                sl, sl, band_t.rearrange("p (h f) -> p h f", h=2)
                            )

                    fillers(p, qb)

                    # ---- PV (transposed): out[q, (j2,hh,65)] per j-pair ----
                    # Four accumulation groups (2 qtiles x 2 heads) share each
                    # psum bank.  A matmul with start=True would zero the
                    # whole 2KB region (clobbering sibling groups), so the
                    # bank was zeroed by the memset above and every matmul
                    # accumulates (start=False).
                    for jp in range(2):
                        jlo, jhi = 2 * jp, 2 * jp + 1
                        pv = pv_t[jp]
                        clast = (4 * qb + jhi) if causal else NT - 1
                        for c in range(clast + 1):
                            for jloc, j in ((0, jlo), (1, jhi)):
                                cg_last = (4 * qb + j) if causal else NT - 1
                                if c > cg_last:
                                    continue
                                for hh in (0, 1):
                                    nc.tensor.matmul(
                                        pv[:, (jloc * 2 + hh) * VE : (jloc * 2 + hh + 1) * VE],
                                        e_grp[:, (2 * c + hh) * F + j * CH : (2 * c + hh) * F + (j + 1) * CH],
                                        vsb[:, c * HPC * VE + (2 * p + hh) * VE : c * HPC * VE + (2 * p + hh + 1) * VE],
                                        start=(c == 0 and jloc == 0 and hh == 0),
                                        stop=(c == cg_last),
                                        skip_group_check=True,
                                    )
                        if jp == 0:
                            normed = npool.tile([128, 512], bf16, tag="normed", name="normed")
                        recip4 = npool.tile([128, 4], f32, tag="recip4", name="recip4")
                        pv4 = pv[:, 0 : 4 * VE].rearrange("p (j h e) -> p j h e", j=2, h=2)
                        with tc.high_priority(offset=400):
                            nc.vector.reciprocal(
                                recip4.rearrange("p (j h o) -> p j h o", j=2, h=2),
                                pv4[:, :, :, DH : DH + 1],
                            )
                            nc.vector.tensor_mul(
                                normed[:, jp * 256 : (jp + 1) * 256].rearrange(
                                    "p (j h e) -> p j h e", j=2, h=2
                                ),
                                pv4[:, :, :, 0:DH],
                                recip4.rearrange("p (j h o) -> p j h o", j=2, h=2)
                                .broadcast_to([128, 2, 2, DH]),
                            )

                    # ---- deferred transposes / w_o from the previous qb ----
                    flush_pending()
                    if p == PAIRS - 1 and qb == NQB - 1:
                        # final tail: emit inline, split by j-pair so the
                        # first transposes overlap the last PV/norm chain
                        make_tail(p, qb, normed, jps=(0,))()
                        make_tail(p, qb, normed, jps=(1,))()
                    else:
                        pending.append(make_tail(p, qb, normed))

            flush_pending()

    _split_excess_waits(nc)
    return nc
